# revision 7
# baseline (speedup 1.0000x reference)
# Additive (Bahdanau) attention Trainium2 kernel — sine-expansion formulation.
#
# Problem shapes (hardcoded): B=4, Tq=256, Tv=1024, D=512, A=128.
#   k = inputs @ Wk + bk                  [B,Tv,A]
#   q = context @ Wq + bq                 [B,Tq,A]
#   scores[b,i,v] = sum_a attn_v[a] * tanh(q[b,i,a] + k[b,v,a]) + (1-mask)*NEG_BIG
#   out = softmax_v(scores) @ inputs      [B,Tq,D]
#
# Sharding: 8 cores = (batch b = c//2) x (query half qh = c%2); each core owns
# 128 queries with the full Tv, so softmax is local and no collectives are
# needed.
#
# Algebraic trick: tanh(x) ~= sum_j beta_j sin(omega_j x) (J=6 fit, Gaussian-
# weighted; end-to-end rel err ~2e-3 vs the 2e-2 gate).  The sine addition
# theorem makes the score separable:
#   sum_a v_a tanh(q_a+k_a)
#     ~= sum_j beta_j sum_a v_a [sin(w_j q_a)cos(w_j k_a)+cos(w_j q_a)sin(w_j k_a)]
# i.e. plain PE matmuls over the a-dimension, replacing the 16.8M-element tanh
# stream (109us of ACT time) with 10 sin/cos/square passes.
#
# Per-harmonic features (z = k or q value; processed as two pieces:
# piece a = k half 0 [P,512], piece b = k half 1 | q [P,640] so work starts
# as soon as each projection half lands):
#  - The HW Sin table is only valid for |arg| <~ 3.55, so:
#  - j=0,1 (w <= 0.8): |w z| <= ~4.2 -> direct Sin(w z); cos via the shared
#    |z| tile: cos(w z) = Sin(-w |z| + pi/2)  (arg stays in-table).
#  - j=2,3: 3-op DVE range reduction to u in [-pi,pi] via fp32 magic-number
#    rounding (t1 = z*(w/2pi)+1.5*2^23; n2p=(t1-M)*2pi; u=(z*w)-n2p), then
#    sin = Sin(u), cos = Sin(-|u|+pi/2).  (measured max err 1.4e-6 on device)
#  - j=4,5 (constrained w4=2*w2, w5=2*w3): double-angle from j=2,3 features:
#    sin2z = 2 sz cz, cos2z = 1-2 sz^2.  With Pj=sz*cz (DVE tt, bf16 2x) and
#    Sj=sz^2 (ACT Square), the score contribution reduces — dropping
#    v-constant terms that softmax ignores — to two matmul terms per half:
#      (-4 vb P_q) . S_k   +   (2 vb - 4 vb S_q) . P_k
#
# Engine split (busy ~16us each; emission order = tile-scheduler priority):
#   PE : bf16 transposes; kq projections (+bias rank-1); f32r/bf16 score
#        matmuls (moving free 512 -> 1 cyc/row); mask rank-1; P^T; output.
#   DVE: reduction chains, |z|/|u|, products, q-feature scaling (bf16 4x),
#        half the PSUM evacuations, softmax recip, output scaling.
#   ACT: 10 sin/cos/square passes + exp (accum_out = sumexp) + the other
#        PSUM evacuations.
# Inputs/context/weights travel as bf16 (halves DMA fill); features bf16;
# reduction chains stay fp32 (magic rounding needs fp32).

import time

import numpy as np

import concourse.bass as bass
import concourse.tile as tile
from concourse import bacc, mybir
from concourse import bass_utils
from concourse.masks import make_identity

P = 128
B, Tq, Tv, D, A = 4, 256, 1024, 512, 128
NCORES = 8
QC = Tq // 2          # queries per core
DC = D // P           # d chunks (4)
VB = Tv // P          # v blocks (8)
NEG_BIG = -1e9

J = 6
BETA = [1.24172983, 0.344084396, 0.129406813, 0.0664233717, 0.0281683798,
        0.00693259933]
OMEGA = [0.260068589, 0.793209915, 1.33508702, 1.88336663, 2.67017404,
         3.76673326]

TWO_PI = float(2.0 * np.pi)
RMAGIC = float(1.5 * 2 ** 23)   # fp32 round-to-nearest forcing constant

F32 = mybir.dt.float32
F32R = mybir.dt.float32r
BF16 = mybir.dt.bfloat16
AF = mybir.ActivationFunctionType
AL = mybir.AluOpType

# piece widths: a = [k half0 | q] (early), b = k half1
WA, WB = 640, 512


def build_nc():
    nc = bacc.Bacc("TRN2", target_bir_lowering=False, debug=False)

    # cin rows: [ctx (128) | inputs (1024)] packed so one issue covers both
    cin_d = nc.dram_tensor("cin", (QC + Tv, D), BF16, kind="ExternalInput")
    wkq_d = nc.dram_tensor("wkq", (D, 2 * A), BF16, kind="ExternalInput")
    # col consts [A, 8]: beta_j*attn_v for j=0..3 | -4vb4 | 2vb4 | -4vb5 | 2vb5
    vb_d = nc.dram_tensor("vbeta", (A, 8), F32, kind="ExternalInput")
    # row consts [1, Tv + A]: negmask row | (bk+bq) row
    rr_d = nc.dram_tensor("rowc", (1, Tv + A), BF16, kind="ExternalInput")
    y_d = nc.dram_tensor("y", (QC, D), F32, kind="ExternalOutput")

    with tile.TileContext(nc) as tc:
        with (
            tc.tile_pool(name="const", bufs=1) as const,
            tc.tile_pool(name="prep", bufs=2) as prep,
            tc.tile_pool(name="qpool", bufs=6) as qpool,
            tc.tile_pool(name="ps_tr", bufs=2, space="PSUM") as ps_tr,
            tc.tile_pool(name="ps_proj", bufs=2, space="PSUM") as ps_proj,
            tc.tile_pool(name="ps_sc", bufs=1, space="PSUM") as ps_sc,
        ):
            # ---- small constants (before DMAs so memsets don't wait) ----
            identf = const.tile([P, P], F32)
            make_identity(nc, identf[:])
            ident = const.tile([P, P], BF16)
            nc.vector.tensor_copy(ident[:], identf[:])
            pio2 = const.tile([P, 1], F32)
            nc.gpsimd.memset(pio2[:], float(np.pi / 2))
            ones1 = const.tile([1, P], BF16)
            nc.gpsimd.memset(ones1[:], 1.0)
            # dummy Sin first so the trig act-table load lands off the
            # critical path (Copy/Square/Sin share one table set)
            scratch = const.tile([P, 1], F32)
            nc.scalar.activation(scratch[:], pio2[:], AF.Sin)

            # ---- loads ----
            cin_re = cin_d.ap().rearrange("(o p) d -> p o d", p=P)
            cin_t = [const.tile([P, 3, D], BF16, name="cin0")] + [
                const.tile([P, 2, D], BF16, name=f"cin{i}") for i in range(1, 4)
            ]
            wkq_sb = const.tile([P, DC, 2 * A], BF16)
            vb_sb = const.tile([P, 8], F32)
            rr_sb = const.tile([1, Tv + A], BF16)
            nc.sync.dma_start(cin_t[0][:], cin_re[:, 0:3, :])
            nc.sync.dma_start(wkq_sb[:], wkq_d.ap().rearrange("(o p) a -> p o a", p=P))
            nc.sync.dma_start(cin_t[1][:], cin_re[:, 3:5, :])
            nc.sync.dma_start(cin_t[2][:], cin_re[:, 5:7, :])
            nc.sync.dma_start(cin_t[3][:], cin_re[:, 7:9, :])
            nc.sync.dma_start(rr_sb[:], rr_d.ap())
            nc.sync.dma_start(vb_sb[:], vb_d.ap())
            neg_row = rr_sb[:, 0:Tv]
            bkq_row = rr_sb[:, Tv : Tv + A]
            wk_sb = wkq_sb[:, :, 0:A]
            wq_sb = wkq_sb[:, :, A : 2 * A]
            ctx_sb = cin_t[0][:, 0, :]

            def inp_vb(vb):
                if vb < 2:
                    return cin_t[0][:, vb + 1, :]
                return cin_t[1 + (vb - 2) // 2][:, vb % 2, :]

            # ---- context transpose -> ctxT [d, q] (ACT evac) ----
            ctxT_sb = const.tile([P, DC, P], BF16)
            trc = ps_tr.tile([P, 1024], BF16, tag="tr")
            for dc in range(DC):
                nc.tensor.transpose(
                    trc[:, dc * P : (dc + 1) * P],
                    ctx_sb[:, dc * P : (dc + 1) * P],
                    ident[:],
                )
            nc.scalar.copy(ctxT_sb[:], trc[:, 0:512])

            # ---- input transposes (bf16, per vb-pair) + projections ----
            inpT_h = [
                const.tile([P, DC, 512], BF16, name=f"inpT{h}") for h in range(2)
            ]
            # kq pieces: a = [k half0 | q] [P,640] (early); b = k half1
            kq_a = const.tile([P, WA], F32)
            kq_b = const.tile([P, WB], F32)

            def emit_tr_pair(pr, on_scalar):
                trv = ps_tr.tile([P, 1024], BF16, tag="tr", name=f"trv{pr}")
                for i in range(2):
                    vb = pr * 2 + i
                    src = inp_vb(vb)
                    for dc in range(DC):
                        nc.tensor.transpose(
                            trv[:, dc * 256 + i * P : dc * 256 + (i + 1) * P],
                            src[:, dc * P : (dc + 1) * P],
                            ident[:],
                        )
                h, off = pr // 2, (pr % 2) * 256
                dst = inpT_h[h][:, :, off : off + 256]
                srcv = trv[:].rearrange("p (c w) -> p c w", w=256)
                if on_scalar:
                    nc.scalar.copy(dst, srcv)
                else:
                    nc.vector.tensor_copy(dst, srcv)

            def emit_kproj(h):
                pk = ps_proj.tile([P, 512], F32, tag="proj", name=f"pk{h}")
                for dc in range(DC):
                    nc.tensor.matmul(
                        pk[:],
                        wk_sb[:, dc, :],
                        inpT_h[h][:, dc, :],
                        start=(dc == 0),
                        stop=(dc == DC - 1),
                    )
                if h == 0:
                    nc.vector.tensor_copy(kq_a[:, 0:512], pk[:])
                else:
                    nc.vector.tensor_copy(kq_b[:], pk[:])

            def emit_qproj():
                pq = ps_proj.tile([P, P], F32, tag="proj", name="pq")
                for dc in range(DC):
                    nc.tensor.matmul(
                        pq[:],
                        wq_sb[:, dc, :],
                        ctxT_sb[:, dc, :],
                        start=(dc == 0),
                        stop=False,
                    )
                # + (bk+bq) broadcast along q: rank-1 ones-row matmul
                nc.tensor.matmul(
                    pq[:], bkq_row, ones1[:], start=False, stop=True,
                    skip_group_check=True,
                )
                nc.vector.tensor_copy(kq_a[:, 512:640], pq[:])

            emit_tr_pair(0, True)
            emit_tr_pair(1, True)
            emit_qproj()
            emit_kproj(0)
            emit_tr_pair(2, True)
            emit_tr_pair(3, False)
            emit_kproj(1)

            kq = {"a": kq_a, "b": kq_b}
            WP = {"a": WA, "b": WB}

            # ---- scores PSUM (accumulated over all harmonics + mask) ----
            scores_h = [
                ps_sc.tile([P, 512], F32, name=f"scores{h}") for h in range(2)
            ]
            nmm = [0, 0]

            def scoremm(h, stat, fa, fb, last=False):
                mov = fa[:, 0:512] if h == 0 else fb[:]
                nc.tensor.matmul(
                    scores_h[h][:],
                    stat,
                    mov,
                    start=(nmm[h] == 0),
                    stop=last,
                    skip_group_check=True,
                )
                nmm[h] += 1

            def qscale(name, srcb, col, col2=None):
                qs = qpool.tile([P, P], BF16, tag="qs", name=name)
                if col2 is None:
                    nc.vector.tensor_scalar_mul(
                        qs[:], srcb[:, 512:640], vb_sb[:, col : col + 1]
                    )
                else:
                    nc.vector.tensor_scalar(
                        qs[:], srcb[:, 512:640],
                        vb_sb[:, col : col + 1], vb_sb[:, col2 : col2 + 1],
                        AL.mult, AL.add,
                    )
                return qs

            def feat_tiles(name, dt=BF16):
                return {p: const.tile([P, WP[p]], dt, name=f"{name}{p}")
                        for p in ("a", "b")}

            # ---- |kq| (shared by j=0,1 cos) ----
            akq = feat_tiles("akq", F32)
            for p in ("a", "b"):
                nc.vector.scalar_tensor_tensor(
                    akq[p][:], kq[p][:], -1.0, kq[p][:], AL.mult, AL.max
                )

            def emit_chain(j):
                u = feat_tiles(f"u{j}", F32)
                au = feat_tiles(f"au{j}", F32)
                for p in ("a", "b"):
                    t1 = prep.tile([P, WP[p]], F32, tag=f"t1{p}", name=f"t1_{j}{p}")
                    nc.vector.tensor_scalar(
                        t1[:], kq[p][:], OMEGA[j] / TWO_PI, RMAGIC, AL.mult, AL.add
                    )
                    n2p = prep.tile([P, WP[p]], F32, tag=f"n2p{p}", name=f"n2p_{j}{p}")
                    nc.vector.tensor_scalar(
                        n2p[:], t1[:], RMAGIC, TWO_PI, AL.subtract, AL.mult
                    )
                    nc.vector.scalar_tensor_tensor(
                        u[p][:], kq[p][:], OMEGA[j], n2p[:], AL.mult, AL.subtract
                    )
                    nc.vector.scalar_tensor_tensor(
                        au[p][:], u[p][:], -1.0, u[p][:], AL.mult, AL.max
                    )
                return u, au

            def emit_direct_feats(j):
                sf = feat_tiles(f"sf{j}")
                cf = feat_tiles(f"cf{j}")
                for p in ("a", "b"):
                    nc.scalar.activation(
                        sf[p][:], kq[p][:], AF.Sin, scale=OMEGA[j]
                    )
                    nc.scalar.activation(
                        cf[p][:], akq[p][:], AF.Sin, bias=pio2[:], scale=-OMEGA[j]
                    )
                return sf, cf

            def emit_chain_feats(j, u, au):
                sf = feat_tiles(f"sf{j}")
                cf = feat_tiles(f"cf{j}")
                for p in ("a", "b"):
                    nc.scalar.activation(sf[p][:], u[p][:], AF.Sin)
                    nc.scalar.activation(
                        cf[p][:], au[p][:], AF.Sin, bias=pio2[:], scale=-1.0
                    )
                return sf, cf

            def emit_jmms(j, sf, cf):
                qs = qscale(f"qs{j}", sf["a"], j)
                qc = qscale(f"qc{j}", cf["a"], j)
                for h in range(2):
                    scoremm(h, qs[:], cf["a"], cf["b"])
                for h in range(2):
                    scoremm(h, qc[:], sf["a"], sf["b"])

            # ---- emission: chains first (lowest DVE priority below evacs),
            # then per-harmonic features/qscales/mms so small ops preempt ----
            u2, au2 = emit_chain(2)
            sf0, cf0 = emit_direct_feats(0)
            emit_jmms(0, sf0, cf0)
            sf1, cf1 = emit_direct_feats(1)
            emit_jmms(1, sf1, cf1)
            u3, au3 = emit_chain(3)
            sf2, cf2 = emit_chain_feats(2, u2, au2)
            emit_jmms(2, sf2, cf2)
            sf3, cf3 = emit_chain_feats(3, u3, au3)
            emit_jmms(3, sf3, cf3)

            sfj = {2: sf2, 3: sf3}
            cfj = {2: cf2, 3: cf3}
            for jj, j in ((4, 2), (5, 3)):
                pr4 = feat_tiles(f"prod{jj}")
                sq4 = feat_tiles(f"sq{jj}")
                for p in ("a", "b"):
                    nc.vector.tensor_tensor(
                        pr4[p][:], sfj[j][p][:], cfj[j][p][:], AL.mult
                    )
                    nc.scalar.activation(sq4[p][:], sfj[j][p][:], AF.Square)
                c0 = 4 + 2 * (jj - 4)
                t2 = qscale(f"t2_{jj}", pr4["a"], c0)          # -4vb * P_q
                t34 = qscale(f"t34_{jj}", sq4["a"], c0, c0 + 1)  # -4vb*S_q+2vb
                for h in range(2):
                    scoremm(h, t2[:], sq4["a"], sq4["b"])
                for h in range(2):
                    scoremm(h, t34[:], pr4["a"], pr4["b"])

            # ---- mask rank-1 rows close the groups; softmax h-major ----
            expP_h = [const.tile([P, 512], BF16, name=f"expP{h}") for h in range(2)]
            sume = const.tile([P, 2], F32)
            pT_h = [const.tile([P, 4, P], BF16, name=f"pT{h}") for h in range(2)]
            for h in range(2):
                nc.tensor.matmul(
                    scores_h[h][:],
                    ones1[:],
                    neg_row[:, h * 512 : (h + 1) * 512],
                    start=False,
                    stop=True,
                    skip_group_check=True,
                )
                nc.scalar.activation(
                    expP_h[h][:], scores_h[h][:], AF.Exp,
                    accum_out=sume[:, h : h + 1],
                )
                trp = ps_tr.tile([P, 1024], BF16, tag="tr", name=f"trp{h}")
                for i in range(4):
                    nc.tensor.transpose(
                        trp[:, i * P : (i + 1) * P],
                        expP_h[h][:, i * P : (i + 1) * P],
                        ident[:],
                    )
                nc.vector.tensor_copy(pT_h[h][:], trp[:, 0:512])
            sumexp = const.tile([P, 1], F32)
            nc.vector.tensor_tensor(
                sumexp[:], sume[:, 0:1], sume[:, 1:2], AL.add
            )
            recip = const.tile([P, 1], F32)
            nc.vector.reciprocal(recip[:], sumexp[:])

            po_d = [
                ps_proj.tile([P, 256], F32, tag="proj", name=f"po{dh}")
                for dh in range(2)
            ]
            out_sb = const.tile([P, D], F32)
            for dh in range(2):
                sl = slice(dh * 256, (dh + 1) * 256)
                for vb in range(VB):
                    nc.tensor.matmul(
                        po_d[dh][:],
                        pT_h[vb // 4][:, vb % 4, :],
                        inp_vb(vb)[:, sl],
                        start=(vb == 0),
                        stop=(vb == VB - 1),
                    )
                if dh == 0:
                    nc.scalar.mul(out_sb[:, sl], po_d[dh][:], recip[:])
                else:
                    nc.vector.tensor_scalar_mul(
                        out_sb[:, sl], po_d[dh][:], recip[:]
                    )
                nc.sync.dma_start(y_d.ap()[:, sl], out_sb[:, sl])

    nc.compile()
    return nc


_NC_CACHE = None


def _get_nc():
    global _NC_CACHE
    if _NC_CACHE is None:
        _NC_CACHE = build_nc()
    return _NC_CACHE


def kernel(inputs, context, mask, Wk, bk, Wq, bq, attn_v):
    import ml_dtypes

    nc = _get_nc()
    f32 = np.float32
    bf16 = ml_dtypes.bfloat16
    wkq = np.concatenate(
        [np.asarray(Wk, dtype=f32), np.asarray(Wq, dtype=f32)], axis=1
    ).astype(bf16)
    av = np.asarray(attn_v, f32)
    beta = np.asarray(BETA, f32)
    vbeta = np.empty((A, 8), f32)
    for j in range(4):
        vbeta[:, j] = beta[j] * av
    vbeta[:, 4] = -4.0 * beta[4] * av
    vbeta[:, 5] = 2.0 * beta[4] * av
    vbeta[:, 6] = -4.0 * beta[5] * av
    vbeta[:, 7] = 2.0 * beta[5] * av
    bkq = (np.asarray(bk, f32) + np.asarray(bq, f32))[None, :]
    in_maps = []
    for c in range(NCORES):
        b, qh = c // 2, c % 2
        negrow = ((1.0 - mask[b].astype(f32)) * NEG_BIG)[None, :]
        cin = np.concatenate(
            [np.asarray(context[b, qh * QC : (qh + 1) * QC]),
             np.asarray(inputs[b])], axis=0,
        ).astype(bf16)
        in_maps.append({
            "cin": np.ascontiguousarray(cin),
            "wkq": np.ascontiguousarray(wkq),
            "vbeta": np.ascontiguousarray(vbeta),
            "rowc": np.ascontiguousarray(
                np.concatenate([negrow, bkq], axis=1)
            ).astype(bf16),
        })
    res = None
    for attempt, delay in enumerate((0, 10, 30)):
        # transient NRT_EXEC_UNIT_UNRECOVERABLE device wedges recover on retry
        if delay:
            time.sleep(delay)
        try:
            res = bass_utils.run_bass_kernel_spmd(
                nc, in_maps, core_ids=list(range(NCORES))
            )
            break
        except Exception:
            if attempt == 2:
                raise
    out = np.empty((B, Tq, D), f32)
    for c in range(NCORES):
        b, qh = c // 2, c % 2
        out[b, qh * QC : (qh + 1) * QC, :] = res.results[c]["y"]
    return out


# revision 8
# speedup vs baseline: 1.0646x; 1.0646x over previous
# Additive (Bahdanau) attention Trainium2 kernel — sine-expansion formulation.
#
# Problem shapes (hardcoded): B=4, Tq=256, Tv=1024, D=512, A=128.
#   k = inputs @ Wk + bk                  [B,Tv,A]
#   q = context @ Wq + bq                 [B,Tq,A]
#   scores[b,i,v] = sum_a attn_v[a] * tanh(q[b,i,a] + k[b,v,a]) + (1-mask)*NEG_BIG
#   out = softmax_v(scores) @ inputs      [B,Tq,D]
#
# Sharding: 8 cores = (batch b = c//2) x (query half qh = c%2); each core owns
# 128 queries with the full Tv, so softmax is local and no collectives are
# needed.
#
# Algebraic trick: tanh(x) ~= sum_j beta_j sin(omega_j x) (J=6 fit, Gaussian-
# weighted; end-to-end rel err ~2e-3 vs the 2e-2 gate).  The sine addition
# theorem makes the score separable:
#   sum_a v_a tanh(q_a+k_a)
#     ~= sum_j beta_j sum_a v_a [sin(w_j q_a)cos(w_j k_a)+cos(w_j q_a)sin(w_j k_a)]
# i.e. plain PE matmuls over the a-dimension, replacing the 16.8M-element tanh
# stream (109us of ACT time) with 10 sin/cos/square passes.
#
# Per-harmonic features (z = k or q value; processed as two pieces:
# piece a = k half 0 [P,512], piece b = k half 1 | q [P,640] so work starts
# as soon as each projection half lands):
#  - The HW Sin table is only valid for |arg| <~ 3.55, so:
#  - j=0,1 (w <= 0.8): |w z| <= ~4.2 -> direct Sin(w z); cos via the shared
#    |z| tile: cos(w z) = Sin(-w |z| + pi/2)  (arg stays in-table).
#  - j=2,3: 3-op DVE range reduction to u in [-pi,pi] via fp32 magic-number
#    rounding (t1 = z*(w/2pi)+1.5*2^23; n2p=(t1-M)*2pi; u=(z*w)-n2p), then
#    sin = Sin(u), cos = Sin(-|u|+pi/2).  (measured max err 1.4e-6 on device)
#  - j=4,5 (constrained w4=2*w2, w5=2*w3): double-angle from j=2,3 features:
#    sin2z = 2 sz cz, cos2z = 1-2 sz^2.  With Pj=sz*cz (DVE tt, bf16 2x) and
#    Sj=sz^2 (ACT Square), the score contribution reduces — dropping
#    v-constant terms that softmax ignores — to two matmul terms per half:
#      (-4 vb P_q) . S_k   +   (2 vb - 4 vb S_q) . P_k
#
# Engine split (busy ~16us each; emission order = tile-scheduler priority):
#   PE : bf16 transposes; kq projections (+bias rank-1); f32r/bf16 score
#        matmuls (moving free 512 -> 1 cyc/row); mask rank-1; P^T; output.
#   DVE: reduction chains, |z|/|u|, products, q-feature scaling (bf16 4x),
#        half the PSUM evacuations, softmax recip, output scaling.
#   ACT: 10 sin/cos/square passes + exp (accum_out = sumexp) + the other
#        PSUM evacuations.
# Inputs/context/weights travel as bf16 (halves DMA fill); features bf16;
# reduction chains stay fp32 (magic rounding needs fp32).

import time

import numpy as np

import concourse.bass as bass
import concourse.tile as tile
from concourse import bacc, mybir
from concourse import bass_utils
from concourse.masks import make_identity

P = 128
B, Tq, Tv, D, A = 4, 256, 1024, 512, 128
NCORES = 8
QC = Tq // 2          # queries per core
DC = D // P           # d chunks (4)
VB = Tv // P          # v blocks (8)
NEG_BIG = -1e9

J = 6
BETA = [1.24172983, 0.344084396, 0.129406813, 0.0664233717, 0.0281683798,
        0.00693259933]
OMEGA = [0.260068589, 0.793209915, 1.33508702, 1.88336663, 2.67017404,
         3.76673326]

TWO_PI = float(2.0 * np.pi)
RMAGIC = float(1.5 * 2 ** 23)   # fp32 round-to-nearest forcing constant

F32 = mybir.dt.float32
F32R = mybir.dt.float32r
BF16 = mybir.dt.bfloat16
AF = mybir.ActivationFunctionType
AL = mybir.AluOpType

# piece widths: a = [k half0 | q] (early), b = k half1
WA, WB = 640, 512


def build_nc():
    nc = bacc.Bacc("TRN2", target_bir_lowering=False, debug=False)

    # cin rows: [ctx (128) | inputs (1024)] packed so one issue covers both
    cin_d = nc.dram_tensor("cin", (QC + Tv, D), BF16, kind="ExternalInput")
    wkq_d = nc.dram_tensor("wkq", (D, 2 * A), BF16, kind="ExternalInput")
    # col consts [A, 8]: beta_j*attn_v for j=0..3 | -4vb4 | 2vb4 | -4vb5 | 2vb5
    vb_d = nc.dram_tensor("vbeta", (A, 8), F32, kind="ExternalInput")
    # row consts [1, Tv + A]: negmask row | (bk+bq) row
    rr_d = nc.dram_tensor("rowc", (1, Tv + A), BF16, kind="ExternalInput")
    y_d = nc.dram_tensor("y", (QC, D), F32, kind="ExternalOutput")

    with tile.TileContext(nc) as tc:
        with (
            tc.tile_pool(name="const", bufs=1) as const,
            tc.tile_pool(name="prep", bufs=2) as prep,
            tc.tile_pool(name="qpool", bufs=6) as qpool,
            tc.tile_pool(name="ps_tr", bufs=2, space="PSUM") as ps_tr,
            tc.tile_pool(name="ps_proj", bufs=2, space="PSUM") as ps_proj,
            tc.tile_pool(name="ps_sc", bufs=1, space="PSUM") as ps_sc,
        ):
            # ---- small constants (before DMAs so memsets don't wait) ----
            identf = const.tile([P, P], F32)
            make_identity(nc, identf[:])
            ident = const.tile([P, P], BF16)
            nc.vector.tensor_copy(ident[:], identf[:])
            pio2 = const.tile([P, 1], F32)
            nc.gpsimd.memset(pio2[:], float(np.pi / 2))
            ones1 = const.tile([1, P], BF16)
            nc.gpsimd.memset(ones1[:], 1.0)
            # dummy Sin first so the trig act-table load lands off the
            # critical path (Copy/Square/Sin share one table set)
            scratch = const.tile([P, 1], F32)
            nc.scalar.activation(scratch[:], pio2[:], AF.Sin)

            # ---- loads ----
            cin_re = cin_d.ap().rearrange("(o p) d -> p o d", p=P)
            cin_t = [const.tile([P, 3, D], BF16, name="cin0")] + [
                const.tile([P, 2, D], BF16, name=f"cin{i}") for i in range(1, 4)
            ]
            wkq_sb = const.tile([P, DC, 2 * A], BF16)
            vb_sb = const.tile([P, 8], F32)
            rr_sb = const.tile([1, Tv + A], BF16)
            nc.sync.dma_start(cin_t[0][:], cin_re[:, 0:3, :])
            nc.sync.dma_start(wkq_sb[:], wkq_d.ap().rearrange("(o p) a -> p o a", p=P))
            nc.sync.dma_start(cin_t[1][:], cin_re[:, 3:5, :])
            nc.sync.dma_start(cin_t[2][:], cin_re[:, 5:7, :])
            nc.sync.dma_start(cin_t[3][:], cin_re[:, 7:9, :])
            nc.sync.dma_start(rr_sb[:], rr_d.ap())
            nc.sync.dma_start(vb_sb[:], vb_d.ap())
            neg_row = rr_sb[:, 0:Tv]
            bkq_row = rr_sb[:, Tv : Tv + A]
            wk_sb = wkq_sb[:, :, 0:A]
            wq_sb = wkq_sb[:, :, A : 2 * A]
            ctx_sb = cin_t[0][:, 0, :]

            def inp_vb(vb):
                if vb < 2:
                    return cin_t[0][:, vb + 1, :]
                return cin_t[1 + (vb - 2) // 2][:, vb % 2, :]

            # ---- context transpose -> ctxT [d, q] (ACT evac) ----
            ctxT_sb = const.tile([P, DC, P], BF16)
            trc = ps_tr.tile([P, 1024], BF16, tag="tr")
            for dc in range(DC):
                nc.tensor.transpose(
                    trc[:, dc * P : (dc + 1) * P],
                    ctx_sb[:, dc * P : (dc + 1) * P],
                    ident[:],
                )
            nc.scalar.copy(ctxT_sb[:], trc[:, 0:512])

            # ---- input transposes (bf16, per vb-pair) + projections ----
            inpT_q = [
                const.tile([P, DC, 256], BF16, name=f"inpTq{i}") for i in range(4)
            ]
            # kq pieces: a = [k half0 | q] [P,640] (early); b = k half1
            kq_a = const.tile([P, WA], F32)
            kq_b = const.tile([P, WB], F32)

            def emit_tr_pair(pr, on_scalar):
                trv = ps_tr.tile([P, 1024], BF16, tag="tr", name=f"trv{pr}")
                for i in range(2):
                    vb = pr * 2 + i
                    src = inp_vb(vb)
                    for dc in range(DC):
                        nc.tensor.transpose(
                            trv[:, dc * 256 + i * P : dc * 256 + (i + 1) * P],
                            src[:, dc * P : (dc + 1) * P],
                            ident[:],
                        )
                dst = inpT_q[pr][:]
                srcv = trv[:].rearrange("p (c w) -> p c w", w=256)
                if on_scalar:
                    nc.scalar.copy(dst, srcv)
                else:
                    nc.vector.tensor_copy(dst, srcv)

            def emit_kproj(h):
                # per-quarter matmul groups so each starts as soon as its
                # pair's transposes are evacuated
                pk = ps_proj.tile([P, 512], F32, tag="proj", name=f"pk{h}")
                for qt in range(2):
                    for dc in range(DC):
                        nc.tensor.matmul(
                            pk[:, qt * 256 : (qt + 1) * 256],
                            wk_sb[:, dc, :],
                            inpT_q[2 * h + qt][:, dc, :],
                            start=(dc == 0),
                            stop=(dc == DC - 1),
                            skip_group_check=True,
                        )
                    dstq = (kq_a[:, qt * 256 : (qt + 1) * 256] if h == 0
                            else kq_b[:, qt * 256 : (qt + 1) * 256])
                    nc.vector.tensor_copy(dstq, pk[:, qt * 256 : (qt + 1) * 256])

            def emit_qproj():
                pq = ps_proj.tile([P, P], F32, tag="proj", name="pq")
                for dc in range(DC):
                    nc.tensor.matmul(
                        pq[:],
                        wq_sb[:, dc, :],
                        ctxT_sb[:, dc, :],
                        start=(dc == 0),
                        stop=False,
                    )
                # + (bk+bq) broadcast along q: rank-1 ones-row matmul
                nc.tensor.matmul(
                    pq[:], bkq_row, ones1[:], start=False, stop=True,
                    skip_group_check=True,
                )
                nc.vector.tensor_copy(kq_a[:, 512:640], pq[:])

            emit_tr_pair(0, False)
            emit_tr_pair(1, False)
            emit_qproj()
            emit_kproj(0)
            emit_tr_pair(2, False)
            emit_tr_pair(3, False)
            emit_kproj(1)

            kq = {"a": kq_a, "b": kq_b}
            WP = {"a": WA, "b": WB}

            # ---- scores PSUM (accumulated over all harmonics + mask) ----
            scores_h = [
                ps_sc.tile([P, 512], F32, name=f"scores{h}") for h in range(2)
            ]
            nmm = [0, 0]

            def scoremm(h, stat, fa, fb, last=False):
                mov = fa[:, 0:512] if h == 0 else fb[:]
                nc.tensor.matmul(
                    scores_h[h][:],
                    stat,
                    mov,
                    start=(nmm[h] == 0),
                    stop=last,
                    skip_group_check=True,
                )
                nmm[h] += 1

            def qscale(name, srcb, col, col2=None):
                qs = qpool.tile([P, P], BF16, tag="qs", name=name)
                if col2 is None:
                    nc.vector.tensor_scalar_mul(
                        qs[:], srcb[:, 512:640], vb_sb[:, col : col + 1]
                    )
                else:
                    nc.vector.tensor_scalar(
                        qs[:], srcb[:, 512:640],
                        vb_sb[:, col : col + 1], vb_sb[:, col2 : col2 + 1],
                        AL.mult, AL.add,
                    )
                return qs

            def feat_tiles(name, dt=BF16):
                return {p: const.tile([P, WP[p]], dt, name=f"{name}{p}")
                        for p in ("a", "b")}

            # ---- |kq| (shared by j=0,1 cos) ----
            akq = feat_tiles("akq", F32)
            for p in ("a", "b"):
                nc.vector.scalar_tensor_tensor(
                    akq[p][:], kq[p][:], -1.0, kq[p][:], AL.mult, AL.max
                )

            def emit_chain(j):
                u = feat_tiles(f"u{j}", F32)
                au = feat_tiles(f"au{j}", F32)
                for p in ("a", "b"):
                    t1 = prep.tile([P, WP[p]], F32, tag=f"t1{p}", name=f"t1_{j}{p}")
                    nc.vector.tensor_scalar(
                        t1[:], kq[p][:], OMEGA[j] / TWO_PI, RMAGIC, AL.mult, AL.add
                    )
                    n2p = prep.tile([P, WP[p]], F32, tag=f"n2p{p}", name=f"n2p_{j}{p}")
                    nc.vector.tensor_scalar(
                        n2p[:], t1[:], RMAGIC, TWO_PI, AL.subtract, AL.mult
                    )
                    nc.vector.scalar_tensor_tensor(
                        u[p][:], kq[p][:], OMEGA[j], n2p[:], AL.mult, AL.subtract
                    )
                    nc.vector.scalar_tensor_tensor(
                        au[p][:], u[p][:], -1.0, u[p][:], AL.mult, AL.max
                    )
                return u, au

            def emit_direct_feats(j):
                sf = feat_tiles(f"sf{j}")
                cf = feat_tiles(f"cf{j}")
                for p in ("a", "b"):
                    nc.scalar.activation(
                        sf[p][:], kq[p][:], AF.Sin, scale=OMEGA[j]
                    )
                    nc.scalar.activation(
                        cf[p][:], akq[p][:], AF.Sin, bias=pio2[:], scale=-OMEGA[j]
                    )
                return sf, cf

            def emit_chain_feats(j, u, au):
                sf = feat_tiles(f"sf{j}")
                cf = feat_tiles(f"cf{j}")
                for p in ("a", "b"):
                    nc.scalar.activation(sf[p][:], u[p][:], AF.Sin)
                    nc.scalar.activation(
                        cf[p][:], au[p][:], AF.Sin, bias=pio2[:], scale=-1.0
                    )
                return sf, cf

            def emit_jmms(j, sf, cf):
                qs = qscale(f"qs{j}", sf["a"], j)
                qc = qscale(f"qc{j}", cf["a"], j)
                for h in range(2):
                    scoremm(h, qs[:], cf["a"], cf["b"])
                for h in range(2):
                    scoremm(h, qc[:], sf["a"], sf["b"])

            # ---- emission: chains first (lowest DVE priority below evacs),
            # then per-harmonic features/qscales/mms so small ops preempt ----
            u2, au2 = emit_chain(2)
            sf0, cf0 = emit_direct_feats(0)
            emit_jmms(0, sf0, cf0)
            sf1, cf1 = emit_direct_feats(1)
            emit_jmms(1, sf1, cf1)
            u3, au3 = emit_chain(3)
            sf2, cf2 = emit_chain_feats(2, u2, au2)
            emit_jmms(2, sf2, cf2)
            sf3, cf3 = emit_chain_feats(3, u3, au3)
            emit_jmms(3, sf3, cf3)

            sfj = {2: sf2, 3: sf3}
            cfj = {2: cf2, 3: cf3}
            for jj, j in ((4, 2), (5, 3)):
                pr4 = feat_tiles(f"prod{jj}")
                sq4 = feat_tiles(f"sq{jj}")
                for p in ("a", "b"):
                    nc.vector.tensor_tensor(
                        pr4[p][:], sfj[j][p][:], cfj[j][p][:], AL.mult
                    )
                    nc.scalar.activation(sq4[p][:], sfj[j][p][:], AF.Square)
                c0 = 4 + 2 * (jj - 4)
                t2 = qscale(f"t2_{jj}", pr4["a"], c0)          # -4vb * P_q
                t34 = qscale(f"t34_{jj}", sq4["a"], c0, c0 + 1)  # -4vb*S_q+2vb
                for h in range(2):
                    scoremm(h, t2[:], sq4["a"], sq4["b"])
                for h in range(2):
                    scoremm(h, t34[:], pr4["a"], pr4["b"])

            # ---- mask rank-1 rows close the groups; softmax h-major ----
            expP_h = [const.tile([P, 512], BF16, name=f"expP{h}") for h in range(2)]
            sume = const.tile([P, 2], F32)
            pT_h = [const.tile([P, 4, P], BF16, name=f"pT{h}") for h in range(2)]
            for h in range(2):
                nc.tensor.matmul(
                    scores_h[h][:],
                    ones1[:],
                    neg_row[:, h * 512 : (h + 1) * 512],
                    start=False,
                    stop=True,
                    skip_group_check=True,
                )
                nc.scalar.activation(
                    expP_h[h][:], scores_h[h][:], AF.Exp,
                    accum_out=sume[:, h : h + 1],
                )
                trp = ps_tr.tile([P, 1024], BF16, tag="tr", name=f"trp{h}")
                for i in range(4):
                    nc.tensor.transpose(
                        trp[:, i * P : (i + 1) * P],
                        expP_h[h][:, i * P : (i + 1) * P],
                        ident[:],
                    )
                nc.vector.tensor_copy(pT_h[h][:], trp[:, 0:512])
            sumexp = const.tile([P, 1], F32)
            nc.vector.tensor_tensor(
                sumexp[:], sume[:, 0:1], sume[:, 1:2], AL.add
            )
            recip = const.tile([P, 1], F32)
            nc.vector.reciprocal(recip[:], sumexp[:])

            po_d = [
                ps_proj.tile([P, 256], F32, tag="proj", name=f"po{dh}")
                for dh in range(2)
            ]
            out_sb = const.tile([P, D], F32)
            for dh in range(2):
                sl = slice(dh * 256, (dh + 1) * 256)
                for vb in range(VB):
                    nc.tensor.matmul(
                        po_d[dh][:],
                        pT_h[vb // 4][:, vb % 4, :],
                        inp_vb(vb)[:, sl],
                        start=(vb == 0),
                        stop=(vb == VB - 1),
                    )
                if dh == 0:
                    nc.scalar.mul(out_sb[:, sl], po_d[dh][:], recip[:])
                else:
                    nc.vector.tensor_scalar_mul(
                        out_sb[:, sl], po_d[dh][:], recip[:]
                    )
                nc.sync.dma_start(y_d.ap()[:, sl], out_sb[:, sl])

            # ---- PE p-state keep-warm: filler transposes, emitted last so
            # they have the highest priority number and only run when the PE
            # would otherwise idle (keeps the clock ramped for real work) ----
            with tc.tile_pool(name="ps_warm", bufs=1, space="PSUM") as ps_warm:
                warm = ps_warm.tile([P, P], BF16, tag="warm")
                for _ in range(140):
                    nc.tensor.transpose(warm[:], ident[:], ident[:])

    nc.compile()
    return nc


_NC_CACHE = None


def _get_nc():
    global _NC_CACHE
    if _NC_CACHE is None:
        _NC_CACHE = build_nc()
    return _NC_CACHE


def kernel(inputs, context, mask, Wk, bk, Wq, bq, attn_v):
    import ml_dtypes

    nc = _get_nc()
    f32 = np.float32
    bf16 = ml_dtypes.bfloat16
    wkq = np.concatenate(
        [np.asarray(Wk, dtype=f32), np.asarray(Wq, dtype=f32)], axis=1
    ).astype(bf16)
    av = np.asarray(attn_v, f32)
    beta = np.asarray(BETA, f32)
    vbeta = np.empty((A, 8), f32)
    for j in range(4):
        vbeta[:, j] = beta[j] * av
    vbeta[:, 4] = -4.0 * beta[4] * av
    vbeta[:, 5] = 2.0 * beta[4] * av
    vbeta[:, 6] = -4.0 * beta[5] * av
    vbeta[:, 7] = 2.0 * beta[5] * av
    bkq = (np.asarray(bk, f32) + np.asarray(bq, f32))[None, :]
    in_maps = []
    for c in range(NCORES):
        b, qh = c // 2, c % 2
        negrow = ((1.0 - mask[b].astype(f32)) * NEG_BIG)[None, :]
        cin = np.concatenate(
            [np.asarray(context[b, qh * QC : (qh + 1) * QC]),
             np.asarray(inputs[b])], axis=0,
        ).astype(bf16)
        in_maps.append({
            "cin": np.ascontiguousarray(cin),
            "wkq": np.ascontiguousarray(wkq),
            "vbeta": np.ascontiguousarray(vbeta),
            "rowc": np.ascontiguousarray(
                np.concatenate([negrow, bkq], axis=1)
            ).astype(bf16),
        })
    res = None
    for attempt, delay in enumerate((0, 10, 30)):
        # transient NRT_EXEC_UNIT_UNRECOVERABLE device wedges recover on retry
        if delay:
            time.sleep(delay)
        try:
            res = bass_utils.run_bass_kernel_spmd(
                nc, in_maps, core_ids=list(range(NCORES))
            )
            break
        except Exception:
            if attempt == 2:
                raise
    out = np.empty((B, Tq, D), f32)
    for c in range(NCORES):
        b, qh = c // 2, c % 2
        out[b, qh * QC : (qh + 1) * QC, :] = res.results[c]["y"]
    return out
